# revision 28
# baseline (speedup 1.0000x reference)
"""Grouped categorical log-softmax (segment logsumexp) on 8 Trainium2 cores.

Final design (driven by per-instruction NTFF profiles of each revision;
65us baseline -> 45us):
  * fp16 device I/O, load + store DMA streams overlapped via a 5-stage
    software pipeline over ~8 bucket-aligned chunks (tiny first chunk
    ramps the pipeline fast; small last chunk shortens the tail).
  * One ACT table load (natural_log_exp_and_others) for both Exp and Ln.
  * Segment slots are padded to multiple-of-4 canonical lengths and each
    slot's elements are interleaved across the four quarters of its
    chunk, so two in-place fp16 tensor_adds at 2x ("folds") cut the 1x
    reduce_sum work to a quarter.
  * Ln writes its result pre-broadcast per slot over the quarter-width
    layout (dense fp16), so the subtract is one whole-chunk dense fp16
    tensor_tensor at 2x.  All chunk quarter-widths are forced even so
    every 16-bit operand stays 4-byte aligned (odd widths silently drop
    the DVE to 1x - measured in v3).
  * All DMA triggers (~700ns each) live on the sync queue; the ACT queue
    only runs exp + ln.

Padding -12 keeps exp() subnormal-positive in fp16 so padded slots give
finite ln.  Length-1 segments are exactly 0 and host-filled; empty
segments produce no output.
"""
from contextlib import ExitStack

import numpy as np

N_CORES = 8
P = 128
PAD_VAL = -12.0          # exp(-12) ~ 6e-6: fp16-subnormal, >0 so ln stays finite
FULL_CAPS = (512, 1536, 3120)  # full columns per chunk: graded ramp (tiny,
                         # medium, then full steady-state chunks) so early
                         # exps never wait behind a large first load
ACT_SET_NL_EXP = 6       # natural_log_exp_and_others in act_info.json


def _canon_lengths(max_len):
    canon = list(range(4, 49, 4)) + [56, 64, 80, 96, 128]
    while canon[-1] < max_len:
        canon.append(canon[-1] * 2)
    return np.asarray(canon, dtype=np.int64)


def _plan_buckets(index, num_segments):
    """Placement plan mapping every element to (core, flat offset) in the
    per-core [128, W] fp16 layout, plus chunk/region metadata.  Chunk
    geometry: full width G = 4*Qw; element i of a slot with padded length
    L (H2 = L/4) lands at column base + (i//H2)*Qw + slot_off + i%H2."""
    S = int(num_segments)
    idx = np.asarray(index).astype(np.int64)
    L = np.bincount(idx, minlength=S)
    starts = np.zeros(S + 1, dtype=np.int64)
    np.cumsum(L, out=starts[1:])

    seg1 = np.where(L == 1)[0]
    sel = np.where(L >= 2)[0]
    plan = dict(seg1=seg1, starts=starts)
    if len(sel) == 0:
        plan.update(W=0, chunks=(), Q_total=0,
                    e_src=np.empty(0, np.int64), e_coreflat=np.empty(0, np.int64))
        return plan
    Ls = L[sel]
    canon = _canon_lengths(int(Ls.max()))
    Lc = canon[np.searchsorted(canon, Ls, side="left")]

    order = np.argsort(Lc, kind="stable")
    segs_sorted = sel[order]
    Ls_sorted = Ls[order]
    Lc_sorted = Lc[order]
    uniq, ustart, ucount = np.unique(Lc_sorted, return_index=True, return_counts=True)

    # --- bucket-aligned chunk construction (quarter coordinates) -------
    chunks = []
    cur_regions, cur_qw = [], 0
    qoff = 0
    bucket_runs = {}

    cur_S = 4

    def cap():
        return FULL_CAPS[min(len(chunks), len(FULL_CAPS) - 1)] // cur_S

    def close_chunk():
        nonlocal cur_regions, cur_qw
        if cur_regions:
            cur_qw += cur_qw & 1  # dead column keeps Qw even (4B alignment)
            chunks.append(dict(Qw=cur_qw, S=cur_S, regions=tuple(cur_regions)))
            cur_regions, cur_qw = [], 0

    binfo = []
    border = sorted(range(len(uniq)), key=lambda i: (int(uniq[i]) % 4 != 0, uniq[i]))
    for bi in border:
        Lb, s0, n = int(uniq[bi]), int(ustart[bi]), int(ucount[bi])
        S = 4 if Lb % 4 == 0 else 2
        c = -(-n // N_CORES)
        q = -(-c // P)
        H2 = Lb // S
        binfo.append((Lb, s0, n, c, q, H2))
        if cur_regions and (S != cur_S or cur_qw + q * H2 > cap()):
            close_chunk()
        cur_S = S
        t0s, cids, rels = [], [], []
        t = 0
        while t < q:
            k = (cap() - cur_qw) // H2
            if k <= 0:
                close_chunk()
                k = max(1, cap() // H2)
            k = min(k, q - t)
            t0s.append(t); cids.append(len(chunks)); rels.append(cur_qw)
            cur_regions.append((cur_qw, k, H2, qoff))
            qoff += k
            cur_qw += k * H2
            t += k
            if cur_qw >= cap():
                close_chunk()
        bucket_runs[Lb] = (np.array(t0s + [q]), np.array(cids), np.array(rels))
    close_chunk()
    Q_total = qoff

    qws = np.array([ch["Qw"] for ch in chunks], dtype=np.int64)
    chS = np.array([ch["S"] for ch in chunks], dtype=np.int64)
    bases = np.zeros(len(chunks) + 1, dtype=np.int64)
    np.cumsum(chS * qws, out=bases[1:])
    W = int(bases[-1])

    qr = []
    q0 = 0
    for ch in chunks:
        nq = sum(r[1] for r in ch["regions"])
        qr.append((q0, q0 + nq))
        q0 += nq

    # --- per-segment placement ----------------------------------------
    nseg = len(segs_sorted)
    seg_core = np.empty(nseg, dtype=np.int64)
    seg_prow = np.empty(nseg, dtype=np.int64)
    seg_col0 = np.empty(nseg, dtype=np.int64)   # base + slot offset in quarter 0
    seg_qw = np.empty(nseg, dtype=np.int64)
    seg_h2 = np.empty(nseg, dtype=np.int64)
    for Lb, s0, n, c, q, H2 in binfo:
        j = np.arange(n)
        core = j // c
        j_loc = j - core * c
        p = j_loc // q
        t = j_loc - p * q
        t0s, cids, rels = bucket_runs[Lb]
        r = np.searchsorted(t0s, t, side="right") - 1
        ch_id = cids[r]
        rel = rels[r] + (t - t0s[r]) * H2
        sl = slice(s0, s0 + n)
        seg_core[sl] = core
        seg_prow[sl] = p
        seg_col0[sl] = bases[ch_id] + rel
        seg_qw[sl] = qws[ch_id]
        seg_h2[sl] = H2

    tot_el = int(Ls_sorted.sum())
    off = np.zeros(nseg + 1, dtype=np.int64)
    np.cumsum(Ls_sorted, out=off[1:])
    within = np.arange(tot_el) - np.repeat(off[:-1], Ls_sorted)
    e_src = np.repeat(starts[segs_sorted], Ls_sorted) + within
    rh2 = np.repeat(seg_h2, Ls_sorted)
    e_col = (np.repeat(seg_col0, Ls_sorted)
             + (within // rh2) * np.repeat(seg_qw, Ls_sorted)
             + within % rh2)
    e_flat = np.repeat(seg_prow, Ls_sorted) * W + e_col
    e_core = np.repeat(seg_core, Ls_sorted)
    plan.update(W=W, Q_total=Q_total, e_src=e_src,
                e_coreflat=e_core * (P * W) + e_flat,
                chunks=tuple((int(b), int(ch["Qw"]), ch["S"], ch["regions"], q01)
                             for ch, b, q01 in zip(chunks, bases[:-1], qr)))
    return plan


def _build_inputs(logits, plan):
    W = plan["W"]
    xin = np.full(N_CORES * P * W, PAD_VAL, dtype=np.float16)
    xin[plan["e_coreflat"]] = np.asarray(logits, dtype=np.float16)[plan["e_src"]]
    return xin.reshape(N_CORES, P * W)


def _gather_output(results_flat, plan, n):
    out = np.zeros(n, dtype=np.float32)
    out[plan["e_src"]] = results_flat.reshape(-1)[plan["e_coreflat"]].astype(np.float32)
    out[plan["starts"][plan["seg1"]]] = 0.0
    return out


def _build_program(W, chunks, Q_total):
    import concourse.bacc as bacc
    import concourse.mybir as mybir
    from concourse import tile

    F16 = mybir.dt.float16
    F32 = mybir.dt.float32
    Exp = mybir.ActivationFunctionType.Exp
    Ln = mybir.ActivationFunctionType.Ln
    AX = mybir.AxisListType.X

    nc = bacc.Bacc("TRN2", target_bir_lowering=False, debug=False,
                   num_devices=N_CORES)
    xin = nc.dram_tensor("xin", [P * W], F16, kind="ExternalInput").ap()
    xout = nc.dram_tensor("xout", [P * W], F16, kind="ExternalOutput").ap()
    xin2d = xin.rearrange("(p w) -> p w", p=P)
    xout2d = xout.rearrange("(p w) -> p w", p=P)

    nc.scalar.add_instruction(mybir.InstLoadActFuncSet(
        name=nc.scalar.bass.get_next_instruction_name(), ins=[], outs=[],
        act_func_set_id=ACT_SET_NL_EXP))

    n = len(chunks)
    qwcap = max(ch[1] for ch in chunks)
    gcap = max(ch[1] * ch[2] for ch in chunks)

    with tile.TileContext(nc) as tc, ExitStack() as ctx:
        xp = ctx.enter_context(tc.tile_pool(name="x", bufs=6))
        ep = ctx.enter_context(tc.tile_pool(name="e", bufs=6))
        cp = ctx.enter_context(tc.tile_pool(name="c", bufs=3))
        sp = ctx.enter_context(tc.tile_pool(name="s", bufs=1))
        st = sp.tile([P, Q_total], F32, tag="st")
        X, E, CE = {}, {}, {}

        with nc.allow_low_precision("fp16 data path by design"):
            # 5-stage pipeline: [load] [exp] [fold+red] [ln-expand] [sub+store]
            for g in range(n + 4):
                if g < n:
                    base, qw, S, _, _ = chunks[g]
                    xt = xp.tile([P, gcap], F16, tag="x")
                    X[g] = xt
                    nc.sync.dma_start(xt[:, :S * qw], xin2d[:, base:base + S * qw])
                if 0 <= g - 1 < n:
                    i = g - 1
                    _, qw, S, _, _ = chunks[i]
                    et = ep.tile([P, gcap], F16, tag="e")
                    E[i] = et
                    nc.scalar.activation(et[:, :S * qw], X[i][:, :S * qw], Exp)
                if 0 <= g - 2 < n:
                    i = g - 2
                    _, qw, S, regions, _ = chunks[i]
                    et = E[i]
                    # in-place folds down to the leading [P, Qw] quarter
                    if S == 4:
                        nc.vector.tensor_add(et[:, :2 * qw], et[:, :2 * qw],
                                             et[:, 2 * qw:4 * qw])
                    nc.vector.tensor_add(et[:, :qw], et[:, :qw],
                                         et[:, qw:2 * qw])
                    for (rel, q, H2, qo) in regions:
                        nc.vector.reduce_sum(
                            st[:, qo:qo + q],
                            et[:, rel:rel + q * H2].rearrange(
                                "p (q h) -> p q h", q=q),
                            axis=AX)
                if 0 <= g - 3 < n:
                    i = g - 3
                    _, qw, S, regions, _ = chunks[i]
                    ce = cp.tile([P, qwcap], F16, tag="c")
                    CE[i] = ce
                    for (rel, q, H2, qo) in regions:
                        nc.scalar.activation(
                            ce[:, rel:rel + q * H2].rearrange(
                                "p (q h) -> p q h", q=q),
                            st[:, qo:qo + q].unsqueeze(2).broadcast_to([P, q, H2]),
                            Ln)
                if 0 <= g - 4 < n:
                    i = g - 4
                    base, qw, S, _, _ = chunks[i]
                    xt, et, ce = X.pop(i), E.pop(i), CE.pop(i)
                    # sub overwrites e (dead after the folds)
                    nc.vector.tensor_sub(
                        et[:, :S * qw].rearrange("p (s h) -> p s h", s=S),
                        xt[:, :S * qw].rearrange("p (s h) -> p s h", s=S),
                        ce[:, :qw].unsqueeze(1).broadcast_to([P, S, qw]))
                    nc.sync.dma_start(xout2d[:, base:base + S * qw],
                                      et[:, :S * qw])
    nc.compile()
    return nc


_cache = {}


def _get_program(plan):
    key = (plan["W"], plan["Q_total"], plan["chunks"])
    if key not in _cache:
        _cache[key] = _build_program(plan["W"], plan["chunks"], plan["Q_total"])
    return _cache[key]


def run_on_device(nc, xin_cores, trace=False, **kw):
    from concourse.bass_utils import run_bass_kernel_spmd
    in_maps = [{"xin": xin_cores[c]} for c in range(N_CORES)]
    res = run_bass_kernel_spmd(nc, in_maps, core_ids=list(range(N_CORES)),
                               trace=trace, **kw)
    out = np.stack([res.results[c]["xout"] for c in range(N_CORES)])
    return out, res


def kernel(logits, index, num_segments):
    logits = np.asarray(logits)
    n = logits.shape[0]
    plan = _plan_buckets(index, num_segments)
    if plan["W"] == 0:
        out = np.zeros(n, dtype=np.float32)
        out[plan["starts"][plan["seg1"]]] = 0.0
        return out
    xin = _build_inputs(logits, plan)
    nc = _get_program(plan)
    out_flat, _ = run_on_device(nc, xin)
    return _gather_output(out_flat, plan, n)


# revision 29
# speedup vs baseline: 1.1623x; 1.1623x over previous
"""Grouped categorical log-softmax (segment logsumexp) on 8 Trainium2 cores.

Final design (driven by per-instruction NTFF profiles of each revision;
65us baseline -> 45us):
  * fp16 device I/O, load + store DMA streams overlapped via a 5-stage
    software pipeline over ~8 bucket-aligned chunks (tiny first chunk
    ramps the pipeline fast; small last chunk shortens the tail).
  * One ACT table load (natural_log_exp_and_others) for both Exp and Ln.
  * Segment slots are padded to multiple-of-4 canonical lengths and each
    slot's elements are interleaved across the four quarters of its
    chunk, so two in-place fp16 tensor_adds at 2x ("folds") cut the 1x
    reduce_sum work to a quarter.
  * Ln writes its result pre-broadcast per slot over the quarter-width
    layout (dense fp16), so the subtract is one whole-chunk dense fp16
    tensor_tensor at 2x.  All chunk quarter-widths are forced even so
    every 16-bit operand stays 4-byte aligned (odd widths silently drop
    the DVE to 1x - measured in v3).
  * All DMA triggers (~700ns each) live on the sync queue; the ACT queue
    only runs exp + ln.

Padding -12 keeps exp() subnormal-positive in fp16 so padded slots give
finite ln.  Length-1 segments are exactly 0 and host-filled; empty
segments produce no output.
"""
from contextlib import ExitStack

import numpy as np

N_CORES = 8
P = 128
PAD_VAL = -12.0          # exp(-12) ~ 6e-6: fp16-subnormal, >0 so ln stays finite
FULL_CAPS = (512, 2368, 3264)  # full columns per chunk: graded ramp (tiny,
                         # medium, then full steady-state chunks) so early
                         # exps never wait behind a large first load; these
                         # values give a runt-free even chunk sequence
ACT_SET_NL_EXP = 6       # natural_log_exp_and_others in act_info.json


def _canon_lengths(max_len):
    canon = list(range(4, 49, 4)) + [56, 64, 80, 96, 128]
    while canon[-1] < max_len:
        canon.append(canon[-1] * 2)
    return np.asarray(canon, dtype=np.int64)


def _plan_buckets(index, num_segments):
    """Placement plan mapping every element to (core, flat offset) in the
    per-core [128, W] fp16 layout, plus chunk/region metadata.  Chunk
    geometry: full width G = 4*Qw; element i of a slot with padded length
    L (H2 = L/4) lands at column base + (i//H2)*Qw + slot_off + i%H2."""
    S = int(num_segments)
    idx = np.asarray(index).astype(np.int64)
    L = np.bincount(idx, minlength=S)
    starts = np.zeros(S + 1, dtype=np.int64)
    np.cumsum(L, out=starts[1:])

    seg1 = np.where(L == 1)[0]
    sel = np.where(L >= 2)[0]
    plan = dict(seg1=seg1, starts=starts)
    if len(sel) == 0:
        plan.update(W=0, chunks=(), Q_total=0,
                    e_src=np.empty(0, np.int64), e_coreflat=np.empty(0, np.int64))
        return plan
    Ls = L[sel]
    canon = _canon_lengths(int(Ls.max()))
    Lc = canon[np.searchsorted(canon, Ls, side="left")]

    order = np.argsort(Lc, kind="stable")
    segs_sorted = sel[order]
    Ls_sorted = Ls[order]
    Lc_sorted = Lc[order]
    uniq, ustart, ucount = np.unique(Lc_sorted, return_index=True, return_counts=True)

    # --- bucket-aligned chunk construction (quarter coordinates) -------
    chunks = []
    cur_regions, cur_qw = [], 0
    qoff = 0
    bucket_runs = {}

    cur_S = 4

    def cap():
        return FULL_CAPS[min(len(chunks), len(FULL_CAPS) - 1)] // cur_S

    def close_chunk():
        nonlocal cur_regions, cur_qw
        if cur_regions:
            cur_qw += cur_qw & 1  # dead column keeps Qw even (4B alignment)
            chunks.append(dict(Qw=cur_qw, S=cur_S, regions=tuple(cur_regions)))
            cur_regions, cur_qw = [], 0

    binfo = []
    border = sorted(range(len(uniq)), key=lambda i: (int(uniq[i]) % 4 != 0, uniq[i]))
    for bi in border:
        Lb, s0, n = int(uniq[bi]), int(ustart[bi]), int(ucount[bi])
        S = 4 if Lb % 4 == 0 else 2
        c = -(-n // N_CORES)
        q = -(-c // P)
        H2 = Lb // S
        binfo.append((Lb, s0, n, c, q, H2))
        if cur_regions and (S != cur_S or cur_qw + q * H2 > cap()):
            close_chunk()
        cur_S = S
        t0s, cids, rels = [], [], []
        t = 0
        while t < q:
            k = (cap() - cur_qw) // H2
            if k <= 0:
                close_chunk()
                k = max(1, cap() // H2)
            k = min(k, q - t)
            t0s.append(t); cids.append(len(chunks)); rels.append(cur_qw)
            cur_regions.append((cur_qw, k, H2, qoff))
            qoff += k
            cur_qw += k * H2
            t += k
            if cur_qw >= cap():
                close_chunk()
        bucket_runs[Lb] = (np.array(t0s + [q]), np.array(cids), np.array(rels))
    close_chunk()
    Q_total = qoff

    qws = np.array([ch["Qw"] for ch in chunks], dtype=np.int64)
    chS = np.array([ch["S"] for ch in chunks], dtype=np.int64)
    bases = np.zeros(len(chunks) + 1, dtype=np.int64)
    np.cumsum(chS * qws, out=bases[1:])
    W = int(bases[-1])

    qr = []
    q0 = 0
    for ch in chunks:
        nq = sum(r[1] for r in ch["regions"])
        qr.append((q0, q0 + nq))
        q0 += nq

    # --- per-segment placement ----------------------------------------
    nseg = len(segs_sorted)
    seg_core = np.empty(nseg, dtype=np.int64)
    seg_prow = np.empty(nseg, dtype=np.int64)
    seg_col0 = np.empty(nseg, dtype=np.int64)   # base + slot offset in quarter 0
    seg_qw = np.empty(nseg, dtype=np.int64)
    seg_h2 = np.empty(nseg, dtype=np.int64)
    for Lb, s0, n, c, q, H2 in binfo:
        j = np.arange(n)
        core = j // c
        j_loc = j - core * c
        p = j_loc // q
        t = j_loc - p * q
        t0s, cids, rels = bucket_runs[Lb]
        r = np.searchsorted(t0s, t, side="right") - 1
        ch_id = cids[r]
        rel = rels[r] + (t - t0s[r]) * H2
        sl = slice(s0, s0 + n)
        seg_core[sl] = core
        seg_prow[sl] = p
        seg_col0[sl] = bases[ch_id] + rel
        seg_qw[sl] = qws[ch_id]
        seg_h2[sl] = H2

    tot_el = int(Ls_sorted.sum())
    off = np.zeros(nseg + 1, dtype=np.int64)
    np.cumsum(Ls_sorted, out=off[1:])
    within = np.arange(tot_el) - np.repeat(off[:-1], Ls_sorted)
    e_src = np.repeat(starts[segs_sorted], Ls_sorted) + within
    rh2 = np.repeat(seg_h2, Ls_sorted)
    e_col = (np.repeat(seg_col0, Ls_sorted)
             + (within // rh2) * np.repeat(seg_qw, Ls_sorted)
             + within % rh2)
    e_flat = np.repeat(seg_prow, Ls_sorted) * W + e_col
    e_core = np.repeat(seg_core, Ls_sorted)
    plan.update(W=W, Q_total=Q_total, e_src=e_src,
                e_coreflat=e_core * (P * W) + e_flat,
                chunks=tuple((int(b), int(ch["Qw"]), ch["S"], ch["regions"], q01)
                             for ch, b, q01 in zip(chunks, bases[:-1], qr)))
    return plan


def _build_inputs(logits, plan):
    W = plan["W"]
    xin = np.full(N_CORES * P * W, PAD_VAL, dtype=np.float16)
    xin[plan["e_coreflat"]] = np.asarray(logits, dtype=np.float16)[plan["e_src"]]
    return xin.reshape(N_CORES, P * W)


def _gather_output(results_flat, plan, n):
    out = np.zeros(n, dtype=np.float32)
    out[plan["e_src"]] = results_flat.reshape(-1)[plan["e_coreflat"]].astype(np.float32)
    out[plan["starts"][plan["seg1"]]] = 0.0
    return out


def _build_program(W, chunks, Q_total):
    import concourse.bacc as bacc
    import concourse.mybir as mybir
    from concourse import tile

    F16 = mybir.dt.float16
    F32 = mybir.dt.float32
    Exp = mybir.ActivationFunctionType.Exp
    Ln = mybir.ActivationFunctionType.Ln
    AX = mybir.AxisListType.X

    nc = bacc.Bacc("TRN2", target_bir_lowering=False, debug=False,
                   num_devices=N_CORES)
    xin = nc.dram_tensor("xin", [P * W], F16, kind="ExternalInput").ap()
    xout = nc.dram_tensor("xout", [P * W], F16, kind="ExternalOutput").ap()
    xin2d = xin.rearrange("(p w) -> p w", p=P)
    xout2d = xout.rearrange("(p w) -> p w", p=P)

    nc.scalar.add_instruction(mybir.InstLoadActFuncSet(
        name=nc.scalar.bass.get_next_instruction_name(), ins=[], outs=[],
        act_func_set_id=ACT_SET_NL_EXP))

    n = len(chunks)
    qwcap = max(ch[1] for ch in chunks)
    gcap = max(ch[1] * ch[2] for ch in chunks)

    with tile.TileContext(nc) as tc, ExitStack() as ctx:
        xp = ctx.enter_context(tc.tile_pool(name="x", bufs=6))
        ep = ctx.enter_context(tc.tile_pool(name="e", bufs=6))
        cp = ctx.enter_context(tc.tile_pool(name="c", bufs=3))
        sp = ctx.enter_context(tc.tile_pool(name="s", bufs=1))
        st = sp.tile([P, Q_total], F32, tag="st")
        X, E, CE = {}, {}, {}

        with nc.allow_low_precision("fp16 data path by design"):
            # 5-stage pipeline: [load] [exp] [fold+red] [ln-expand] [sub+store]
            for g in range(n + 4):
                if g < n:
                    base, qw, S, _, _ = chunks[g]
                    xt = xp.tile([P, gcap], F16, tag="x")
                    X[g] = xt
                    nc.sync.dma_start(xt[:, :S * qw], xin2d[:, base:base + S * qw])
                if 0 <= g - 1 < n:
                    i = g - 1
                    _, qw, S, _, _ = chunks[i]
                    et = ep.tile([P, gcap], F16, tag="e")
                    E[i] = et
                    nc.scalar.activation(et[:, :S * qw], X[i][:, :S * qw], Exp)
                if 0 <= g - 2 < n:
                    i = g - 2
                    _, qw, S, regions, _ = chunks[i]
                    et = E[i]
                    # in-place folds down to the leading [P, Qw] quarter
                    if S == 4:
                        nc.vector.tensor_add(et[:, :2 * qw], et[:, :2 * qw],
                                             et[:, 2 * qw:4 * qw])
                    nc.vector.tensor_add(et[:, :qw], et[:, :qw],
                                         et[:, qw:2 * qw])
                    for (rel, q, H2, qo) in regions:
                        nc.vector.reduce_sum(
                            st[:, qo:qo + q],
                            et[:, rel:rel + q * H2].rearrange(
                                "p (q h) -> p q h", q=q),
                            axis=AX)
                if 0 <= g - 3 < n:
                    i = g - 3
                    _, qw, S, regions, _ = chunks[i]
                    ce = cp.tile([P, qwcap], F16, tag="c")
                    CE[i] = ce
                    for (rel, q, H2, qo) in regions:
                        nc.scalar.activation(
                            ce[:, rel:rel + q * H2].rearrange(
                                "p (q h) -> p q h", q=q),
                            st[:, qo:qo + q].unsqueeze(2).broadcast_to([P, q, H2]),
                            Ln)
                if 0 <= g - 4 < n:
                    i = g - 4
                    base, qw, S, _, _ = chunks[i]
                    xt, et, ce = X.pop(i), E.pop(i), CE.pop(i)
                    # sub overwrites e (dead after the folds)
                    nc.vector.tensor_sub(
                        et[:, :S * qw].rearrange("p (s h) -> p s h", s=S),
                        xt[:, :S * qw].rearrange("p (s h) -> p s h", s=S),
                        ce[:, :qw].unsqueeze(1).broadcast_to([P, S, qw]))
                    nc.sync.dma_start(xout2d[:, base:base + S * qw],
                                      et[:, :S * qw])
    nc.compile()
    return nc


_cache = {}


def _get_program(plan):
    key = (plan["W"], plan["Q_total"], plan["chunks"])
    if key not in _cache:
        _cache[key] = _build_program(plan["W"], plan["chunks"], plan["Q_total"])
    return _cache[key]


def run_on_device(nc, xin_cores, trace=False, **kw):
    from concourse.bass_utils import run_bass_kernel_spmd
    in_maps = [{"xin": xin_cores[c]} for c in range(N_CORES)]
    res = run_bass_kernel_spmd(nc, in_maps, core_ids=list(range(N_CORES)),
                               trace=trace, **kw)
    out = np.stack([res.results[c]["xout"] for c in range(N_CORES)])
    return out, res


def kernel(logits, index, num_segments):
    logits = np.asarray(logits)
    n = logits.shape[0]
    plan = _plan_buckets(index, num_segments)
    if plan["W"] == 0:
        out = np.zeros(n, dtype=np.float32)
        out[plan["starts"][plan["seg1"]]] = 0.0
        return out
    xin = _build_inputs(logits, plan)
    nc = _get_program(plan)
    out_flat, _ = run_on_device(nc, xin)
    return _gather_output(out_flat, plan, n)
